# revision 1
# baseline (speedup 1.0000x reference)
"""Locally-connected layer (3x3, stride 1) on 8 TRN2 NeuronCores — v6.

Math: out[b,o,i,j] = sum_{c,kh,kw} x[b,c,i+kh,j+kw] * W[c,o,i,j,kh,kw] + bias[o,i,j]
  x: [128, 64, 32, 32] f32, W: [64, 64, 30, 30, 3, 3] f32, bias: [64, 30, 30] f32
  out: [128, 64, 30, 30] f32

Sharding: each core owns 4 output rows (cores 6,7 overlap rows 24-27/26-29;
host keeps rows 28-29 from core 7).

v6 minimizes PE instruction count (HW is sequencer/LDWEIGHTS-bound, ~85ns per
PE instruction vs 26.7ns of streaming per N=64 matmul):
 - Two kh-taps per matmul: x pixel rows (1,2) and (3,4) are stacked along the
   128-partition contraction dim (xp). Row pair (1,2) covers i=0 (taps 1,2)
   and i=1 (taps 0,1); pair (3,4) covers i=2 (taps 1,2) and i=3 (taps 0,1).
   Leftover single taps: kh=0 for even i (pixel rows 0,2), kh=2 for odd i
   (pixel rows 3,5), contracting K=64 (xs).
 - All 3 kw-taps of one (stationary, output row) go into ONE matmul: for
   stationary pixel column w, the taps (j=w-2,kw=2),(j=w-1,kw=1),(j=w,kw=0)
   write three adjacent 64-col j-blocks of the same PSUM tile, and in the
   (j, i, kw, o) weight layout their blocks sit at constant stride 704, so a
   3D strided AP streams up to N=192 in one instruction.
 - Per pixel column w: 6 LDWEIGHTS + 8 matmuls (was 7 + 24 one-tap matmuls).
Bias is added by a K=1 ones-row matmul per psum tile, which is also the single
start=True matmul: start=True clears has_written for the WHOLE psum bank, so
it must precede every accumulating matmul into that bank.

Output path: PSUM -> bf16 staging (DVE cast) -> one DMA per band to DRAM laid
out [B, R, OW, O] (contiguous per partition), on the scalar HWDGE ring so it
never blocks weight prefetch on the SP ring. Host transposes to [B,O,OH,OW].
For_i ends every iteration with an all-engine barrier, so `repeat` unrolls 2
bodies per iteration to amortize it.
"""

import sys

for _p in ("/opt/trn_rl_repo",):
    if _p not in sys.path:
        sys.path.insert(0, _p)

import numpy as np
import ml_dtypes

import concourse.bass as bass
import concourse.tile as tile
from concourse import bacc, mybir
from concourse.bass_utils import run_bass_kernel_spmd

N_CORES = 8
B = 128
C = 64          # contracted channel dim (weight axis 0)
O = 64          # output channel dim (weight axis 1)
H = 32
W = 32
K = 3
OH = 30
OW = 30
R = 4           # output rows per core
H6 = R + K - 1  # input rows per core
ROW0 = [0, 4, 8, 12, 16, 20, 24, 26]  # first output row per core
BANDS = [(0, 8), (8, 8), (16, 8), (24, 6)]  # (j0, width) PSUM j-bands

NPAIR = 2                      # pair stationaries: pixel rows (1,2),(3,4)
XP_FREE = NPAIR * W * B        # 8192
XS_FREE = 2 * W * B            # singles: 2 slots x 2 partition halves
WTP_PER_J = R * K * O          # 768: (i, kw, o)  [wp]
WS_PER_J = 2 * K * O           # 384: (iH, kw, o) [ws, per partition half]
WB_PER_J = R * O               # 256: (i, o)      [wb bias row]
WP_FREE = OW * WTP_PER_J       # 23040
WS_FREE = OW * WS_PER_J        # 11520
WB_FREE = OW * WB_PER_J        # 7680
STRIDE_DP = WTP_PER_J - O      # 704: wp (j+1, kw-1) block stride
STRIDE_DS = WS_PER_J - O       # 320: ws (j+1, kw-1) block stride
WPAD = STRIDE_DP               # tile padding so strided rearrange stays in-bounds

UNROLL = 4                                # bodies per For_i iteration
PAIR_IS = {0: (0, 1), 1: (2, 3)}          # hp -> output rows using that pair
# singles: even i -> partition half 0 (pixel rows 0,2), odd i -> half 1
# (rows 3,5); alternating halves lets the PE pull the next LDWEIGHTS ahead
# (disjoint row groups) instead of serializing LDW->MM->LDW->MM.
SING_ROWS_H0 = [0, 2]                     # xs[0:64, slot]  -> i = 0, 2
SING_ROWS_H1 = [3, 5]                     # xs[64:128, slot]-> i = 1, 3

_BF16 = ml_dtypes.bfloat16


def _kha(i):  # tap kh of the pair's FIRST pixel row
    return 1 if i % 2 == 0 else 0


def _khs(i):  # single tap kh
    return 0 if i % 2 == 0 else 2


def build_nc(repeat: int = 1, mode: str = "full"):
    nc = bacc.Bacc("TRN2", target_bir_lowering=False, debug=False,
                   num_devices=N_CORES)
    xp_ap = nc.dram_tensor("xp", [2 * C, XP_FREE], mybir.dt.bfloat16,
                           kind="ExternalInput").ap()
    xs_ap = nc.dram_tensor("xs", [2 * C, XS_FREE], mybir.dt.bfloat16,
                           kind="ExternalInput").ap()
    wp_ap = nc.dram_tensor("wp", [2 * C, WP_FREE], mybir.dt.bfloat16,
                           kind="ExternalInput").ap()
    ws_ap = nc.dram_tensor("ws", [2 * C, WS_FREE], mybir.dt.bfloat16,
                           kind="ExternalInput").ap()
    wb_ap = nc.dram_tensor("wb", [1, WB_FREE], mybir.dt.bfloat16,
                           kind="ExternalInput").ap()
    ones_ap_d = nc.dram_tensor("onest", [1, B], mybir.dt.bfloat16,
                               kind="ExternalInput").ap()
    out_ap = nc.dram_tensor("outp", [B, R, OW, O], mybir.dt.bfloat16,
                            kind="ExternalOutput").ap()

    with tile.TileContext(nc) as tc:
        with (
            tc.tile_pool(name="xpool", bufs=1) as xpool,
            tc.tile_pool(name="wpool", bufs=3) as wpool,
            tc.tile_pool(name="ppool", bufs=8, space="PSUM") as ppool,
            tc.tile_pool(name="opool", bufs=2) as opool,
        ):
            xp_sb = xpool.tile([2 * C, XP_FREE], mybir.dt.bfloat16, tag="xp")
            xs_sb = xpool.tile([2 * C, XS_FREE], mybir.dt.bfloat16, tag="xs")
            ones_sb = xpool.tile([1, B], mybir.dt.bfloat16, tag="ones")
            nc.sync.dma_start(ones_sb[:, :], ones_ap_d[:, :])
            for q in range(2):
                ch = XP_FREE // 2
                nc.sync.dma_start(xp_sb[:, q * ch:(q + 1) * ch],
                                  xp_ap[:, q * ch:(q + 1) * ch])
            for q in range(2):
                ch = XS_FREE // 2
                nc.sync.dma_start(xs_sb[:, q * ch:(q + 1) * ch],
                                  xs_ap[:, q * ch:(q + 1) * ch])
            xp3 = xp_sb[:].rearrange("p (f b) -> p f b", b=B)
            xs3 = xs_sb[:].rearrange("p (f b) -> p f b", b=B)

            def diag_rhs(sb, p0, p1, per_j, blk, w, j0, jl_lo, n):
                """Moving operand for the kw-batched matmul: n blocks of 64
                cols at stride per_j-64 covering j = j0+jl_lo .. +n-1 with
                kw = w - j."""
                stride = per_j - O
                off0 = jl_lo * per_j + blk * (K * O) \
                    + (w - j0 - jl_lo) * O
                if n == 1:
                    return sb[p0:p1, off0:off0 + O]
                return sb[p0:p1, off0:off0 + n * stride].rearrange(
                    "p (n q) -> p n q", q=stride)[:, :, 0:O]

            def body():
                if mode == "empty":
                    return
                for (j0, bw) in BANDS:
                    wp_sb = wpool.tile([2 * C, BANDS[0][1] * WTP_PER_J + WPAD],
                                       mybir.dt.bfloat16, tag="wp")
                    ws_sb = wpool.tile([2 * C, BANDS[0][1] * WS_PER_J + WPAD],
                                       mybir.dt.bfloat16, tag="ws")
                    wb_sb = wpool.tile([1, BANDS[0][1] * WB_PER_J],
                                       mybir.dt.bfloat16, tag="wb")
                    nc.sync.dma_start(
                        wb_sb[:, 0:bw * WB_PER_J],
                        wb_ap[:, j0 * WB_PER_J:(j0 + bw) * WB_PER_J])
                    for q in range(2):
                        for src, dstt, per_j in ((ws_ap, ws_sb, WS_PER_J),
                                                 (wp_ap, wp_sb, WTP_PER_J)):
                            half = bw * per_j // 2
                            nc.sync.dma_start(
                                dstt[:, q * half:(q + 1) * half],
                                src[:, j0 * per_j + q * half:
                                    j0 * per_j + (q + 1) * half])

                    ps = [ppool.tile([B, 512], mybir.dt.float32, tag="ps",
                                     name=f"ps{i}")
                          for i in range(R)]
                    wb3 = wb_sb[:, 0:bw * WB_PER_J].rearrange(
                        "p (j r) -> p j r", r=WB_PER_J)
                    # bias + has_written priming: start=True clears the WHOLE
                    # psum bank, so it must be the single first matmul per
                    # ps[i]: ones[1,128]^T @ bias[1, bw, 64]
                    for i in range(R):
                        bias_rhs = wb3[0:1, 0:bw, i * O:(i + 1) * O]
                        nc.tensor.matmul(ps[i][:, 0:bw * O], ones_sb[:, :],
                                         bias_rhs,
                                         start=True, stop=(mode == "dma"))
                    w_last = j0 + bw + 1
                    for w in (() if mode == "dma"
                              else range(j0, min(j0 + bw + 2, W))):
                        jl_lo = max(0, w - j0 - 2)
                        jl_hi = min(bw - 1, w - j0)
                        n = jl_hi - jl_lo + 1
                        if n <= 0:
                            continue
                        # singles: K=64, one kw-batched matmul per output
                        # row; i-parity picks the partition half so
                        # consecutive LDWEIGHTS hit disjoint PE row groups
                        for i in (0, 1, 2, 3):
                            p0 = 0 if i % 2 == 0 else C
                            lhs = xs3[p0:p0 + C, (i // 2) * W + w, :]
                            nc.tensor.matmul(
                                ps[i][:, jl_lo * O:(jl_hi + 1) * O],
                                lhs,
                                diag_rhs(ws_sb, p0, p0 + C, WS_PER_J, i // 2,
                                         w, j0, jl_lo, n),
                                start=False, stop=False)
                        # pairs: K=128, one kw-batched matmul per output row
                        for hp in range(NPAIR):
                            lhs = xp3[0:2 * C, hp * W + w, :]
                            for i in PAIR_IS[hp]:
                                nc.tensor.matmul(
                                    ps[i][:, jl_lo * O:(jl_hi + 1) * O],
                                    lhs,
                                    diag_rhs(wp_sb, 0, 2 * C, WTP_PER_J, i,
                                             w, j0, jl_lo, n),
                                    start=False, stop=(w == w_last))
                    # cast to bf16 staging split across DVE (rows 0,1) and
                    # ACT (rows 2,3) so the copies run in parallel and each
                    # half's out DMA starts after 2 copies instead of 4
                    obA = opool.tile([B, 2 * BANDS[0][1] * O],
                                     mybir.dt.bfloat16, tag="obA")
                    obB = opool.tile([B, 2 * BANDS[0][1] * O],
                                     mybir.dt.bfloat16, tag="obB")
                    for i in range(2):
                        nc.vector.tensor_copy(
                            obA[:, i * bw * O:(i + 1) * bw * O],
                            ps[i][:, 0:bw * O])
                        nc.scalar.copy(
                            obB[:, i * bw * O:(i + 1) * bw * O],
                            ps[i + 2][:, 0:bw * O])
                    # scalar ring (qActDynamicHW): keeps the out DMAs off the
                    # SP HWDGE FIFO so weight prefetch never queues behind them
                    for ob, i0 in ((obA, 0), (obB, 2)):
                        nc.scalar.dma_start(
                            out_ap[:, i0:i0 + 2, j0:j0 + bw, :],
                            ob[:, 0:2 * bw * O].rearrange(
                                "p (i j o) -> p i j o", i=2, j=bw))

            # For_i ends every iteration with an all-engine barrier + sem
            # reset (~9us, no cross-iteration pipelining), so unroll UNROLL
            # bodies per iteration to amortize it and overlap each body's
            # tail with the next body's weight prefetch.
            n_loop, rem = divmod(repeat, UNROLL)
            if n_loop <= 1:
                for _ in range(repeat):
                    body()
            else:
                for _ in range(rem):
                    body()
                with tc.For_i(0, n_loop, 1):
                    for _ in range(UNROLL):
                        body()

    nc.compile()
    dedup_ldweights(nc)
    return nc


def dedup_ldweights(nc):
    """Remove consecutive InstLdweights with identical weight APs from the PE
    stream (post-compile)."""
    removed = 0
    for blk in nc.m.functions[0].blocks:
        insts = list(blk.instructions)
        has_pe = any(type(i).__name__ == "InstLdweights" for i in insts)
        if not has_pe:
            continue
        prev_key = None
        to_remove = []
        for inst in insts:
            nm = type(inst).__name__
            if nm == "InstLdweights":
                key = repr(inst.ins[0])
                si = inst.sync_info
                clean = not si or (not si.on_wait and not si.on_update)
                if key == prev_key and clean:
                    to_remove.append(inst)
                else:
                    prev_key = key
            elif nm == "InstMatmult":
                pass
            elif nm in ("InstEventSemaphore", "InstNop", "InstTensorLoad",
                        "InstTensorSave"):
                pass
            else:
                prev_key = None
        for inst in to_remove:
            blk.instructions.remove(inst)
            removed += 1
    return removed


def prep_inputs(x: np.ndarray, weight: np.ndarray, bias: np.ndarray):
    """Host-side shard + relayout + bf16 cast. Returns in_maps for 8 cores."""
    in_maps = []
    x_t = x.transpose(1, 2, 3, 0)            # [C, H, W, B]
    for r0 in ROW0:
        # xp[c + 64*s, hp, w, b] = x[b, c, r0 + 1 + 2*hp + s, w]
        xp = np.empty((2 * C, NPAIR, W, B), dtype=_BF16)
        for s in range(2):
            rows = [r0 + 1 + 2 * hp + s for hp in range(NPAIR)]
            xp[s * C:(s + 1) * C] = x_t[:, rows, :, :].astype(_BF16)
        # xs[half*64 + c, slot, w, b]: half 0 = pixel rows 0,2 (i=0,2);
        # half 1 = rows 3,5 (i=1,3)
        xs = np.empty((2 * C, 2, W, B), dtype=_BF16)
        xs[:C] = x_t[:, [r0 + r for r in SING_ROWS_H0], :, :].astype(_BF16)
        xs[C:] = x_t[:, [r0 + r for r in SING_ROWS_H1], :, :].astype(_BF16)
        # wp[c + 64*s, j, i, kw, o] = weight[c, o, r0+i, j, khA(i)+s, kw]
        # ws[half*64 + c, j, iH, kw, o] = weight[c, o, r0+2*iH+half, j, khS, kw]
        ww = weight[:, :, r0:r0 + R]          # [C, O, R, OW, K, K]
        wp = np.empty((2 * C, OW, R, K, O), dtype=_BF16)
        ws = np.empty((2 * C, OW, 2, K, O), dtype=_BF16)
        for i in range(R):
            khA, khS = _kha(i), _khs(i)
            for s in range(2):
                wp[s * C:(s + 1) * C, :, i] = \
                    ww[:, :, i, :, khA + s, :].transpose(0, 2, 3, 1).astype(_BF16)
            half, iH = i % 2, i // 2
            ws[half * C:(half + 1) * C, :, iH] = \
                ww[:, :, i, :, khS, :].transpose(0, 2, 3, 1).astype(_BF16)
        wb = bias[:, r0:r0 + R, :].transpose(2, 1, 0).astype(_BF16)  # [OW,R,O]
        in_maps.append({
            "xp": np.ascontiguousarray(xp.reshape(2 * C, XP_FREE)),
            "xs": np.ascontiguousarray(xs.reshape(2 * C, XS_FREE)),
            "wp": np.ascontiguousarray(wp.reshape(2 * C, WP_FREE)),
            "ws": np.ascontiguousarray(ws.reshape(2 * C, WS_FREE)),
            "wb": np.ascontiguousarray(wb.reshape(1, WB_FREE)),
            "onest": np.ones((1, B), dtype=_BF16),
        })
    return in_maps


def gather_output(results):
    out = np.empty((B, O, OH, OW), dtype=np.float32)
    for k, r0 in enumerate(ROW0):
        co = np.asarray(results[k]["outp"])               # [B, R, OW, O] bf16
        lo = 0 if k < 7 else 2                            # core 7: keep rows 28-29
        out[:, :, r0 + lo:r0 + R, :] = \
            co[:, lo:].astype(np.float32).transpose(0, 3, 1, 2)
    return out


_NC_CACHE = {}


def kernel(x: np.ndarray, weight: np.ndarray, bias: np.ndarray) -> np.ndarray:
    if "nc" not in _NC_CACHE:
        _NC_CACHE["nc"] = build_nc()
    nc = _NC_CACHE["nc"]
    in_maps = prep_inputs(np.asarray(x), np.asarray(weight), np.asarray(bias))
    res = run_bass_kernel_spmd(nc, in_maps, core_ids=list(range(N_CORES)))
    return gather_output(res.results)

